# revision 13
# baseline (speedup 1.0000x reference)
"""Trainium2 Bass kernel for nn_Decoder (LSTM decoder with SE/HP MLP heads).

Strategy: pure data parallelism over batch (2048 -> 8 cores x 256).
Feature-major on-chip layout ([feature, batch]); weights stationary, batch
on the matmul moving dim. The SE MLP's output projection is folded into the
gate weights on the host (w2t = se_w2 @ w_ih.T), shrinking the x-part
contraction from K=512 to K=16, and all biases are folded into two extra
bf16 contraction rows (hi/lo split) so the gate bias is free.

Per step (32 sequential steps):
  u = relu(A1.T @ lp + c1_se)                      [16, 256]   (f32r matmul)
  gates = w_hh.T-tiles @ h_bf + W2T_ext @ u_ext    [2048, 256] (bf16, PSUM fp32)
  i,f,o = sigmoid(gates), g = tanh(gates)          (ACT, straight from PSUM)
  c = f*c + i*g; h = o*tanh(c)                     (DVE fp32, h also cast bf16)
  v = relu(B1.T @ h + c1_hp); p = hp_w2.T @ v + lp (f32r)
  lp = sigmoid(p + hp_b2)  -> traj[t]
"""

import json

import numpy as np
import ml_dtypes
from contextlib import ExitStack

import concourse.bass as bass
import concourse.mybir as mybir
import concourse.tile as tile
from concourse.bass import ts


def _fix_multiwait(bir_bytes: bytes) -> bytes:
    """Hoist excess sync waits onto injected EventSemaphore carriers
    (HW cap: 2 waits on EventSemaphore, 1 elsewhere; the Tile end-of-kernel
    drain can exceed this and the compiler rejects it)."""
    bir = json.loads(bir_bytes)
    for fn in bir.get("functions", []):
        for blk in fn.get("blocks", []):
            insts = blk.get("instructions")
            if not insts:
                continue
            out = []
            for inst in insts:
                si = inst.get("sync_info")
                waits = (si or {}).get("on_wait") or []
                cap = 2 if inst.get("opcode") == "EventSemaphore" else 1
                if len(waits) > cap:
                    excess, keep = waits[:-cap], waits[-cap:]
                    si["on_wait"] = keep
                    for i in range(0, len(excess), 2):
                        out.append({
                            "debug": inst.get("debug", 0),
                            "engine": inst["engine"],
                            "ins": [],
                            "name": f"{inst['name']}_xw{i}",
                            "opcode": "EventSemaphore",
                            "outs": [],
                            "sync_info": {"on_update": [], "on_wait": excess[i : i + 2]},
                        })
                out.append(inst)
            blk["instructions"] = out
    return json.dumps(bir).encode()

BF16 = ml_dtypes.bfloat16
F32 = np.float32

SEQ = 32
B = 2048
H = 512
E = 512
HID = 16
NCORES = 8
BL = B // NCORES  # 256 local batch
NG = 4 * H  # 2048 gate features
BN_EPS = 1e-5

_CACHE: dict = {}


def _build_nc():
    nc = bass.Bass()
    dt = mybir.dt
    ACTF = mybir.ActivationFunctionType

    # --- DRAM tensors (per-core inputs; weights replicated across cores) ---
    whh_d = nc.dram_tensor("whhT", [4, 128, NG], dt.bfloat16, kind="ExternalInput")
    w2t_d = nc.dram_tensor("w2t", [HID + 2, NG], dt.bfloat16, kind="ExternalInput")
    b1_d = nc.dram_tensor("b1", [4, 128, HID], dt.bfloat16, kind="ExternalInput")
    a1_d = nc.dram_tensor("a1", [2, HID], dt.bfloat16, kind="ExternalInput")
    hpw2_d = nc.dram_tensor("hpw2", [HID, 2], dt.bfloat16, kind="ExternalInput")
    c1se_d = nc.dram_tensor("c1se", [HID, 1], dt.float32, kind="ExternalInput")
    c1hp_d = nc.dram_tensor("c1hp", [HID, 1], dt.float32, kind="ExternalInput")
    hpb2_d = nc.dram_tensor("hpb2", [2, 1], dt.float32, kind="ExternalInput")
    ones2_d = nc.dram_tensor("ones2", [2, BL], dt.bfloat16, kind="ExternalInput")
    lp0b_d = nc.dram_tensor("lp0b", [2, BL], dt.bfloat16, kind="ExternalInput")
    lp0f_d = nc.dram_tensor("lp0f", [2, BL], dt.float32, kind="ExternalInput")
    h0b_d = nc.dram_tensor("h0b", [2, 128, 512], dt.bfloat16, kind="ExternalInput")
    c0_d = nc.dram_tensor("c0", [2, 128, 512], dt.float32, kind="ExternalInput")
    traj_d = nc.dram_tensor("traj", [2, SEQ, BL], dt.float32, kind="ExternalOutput")

    with tile.TileContext(nc) as tc:
        with ExitStack() as ctx:
            singles = ctx.enter_context(tc.tile_pool(name="singles", bufs=1))
            gpool = ctx.enter_context(tc.tile_pool(name="gates", bufs=3))
            psg = ctx.enter_context(tc.tile_pool(name="psg", bufs=5, space="PSUM"))
            pss = ctx.enter_context(tc.tile_pool(name="pss", bufs=1, space="PSUM"))

            # persistent weights
            whh = []
            for k in range(4):
                wt = singles.tile([128, NG], dt.bfloat16, tag=f"whh{k}", name=f"whh{k}")
                nc.sync.dma_start(out=wt, in_=whh_d[k, :, :])
                whh.append(wt)
            w2t = singles.tile([HID + 2, NG], dt.bfloat16)
            nc.sync.dma_start(out=w2t, in_=w2t_d[:, :])
            b1 = []
            for k in range(4):
                bt = singles.tile([128, HID], dt.bfloat16, tag=f"b1{k}", name=f"b1{k}")
                nc.sync.dma_start(out=bt, in_=b1_d[k, :, :])
                b1.append(bt)
            a1 = singles.tile([2, HID], dt.bfloat16)
            nc.sync.dma_start(out=a1, in_=a1_d[:, :])
            hpw2 = singles.tile([HID, 2], dt.bfloat16)
            nc.sync.dma_start(out=hpw2, in_=hpw2_d[:, :])
            c1se = singles.tile([HID, 1], dt.float32)
            nc.sync.dma_start(out=c1se, in_=c1se_d[:, :])
            c1hp = singles.tile([HID, 1], dt.float32)
            nc.sync.dma_start(out=c1hp, in_=c1hp_d[:, :])
            hpb2 = singles.tile([2, 1], dt.float32)
            nc.sync.dma_start(out=hpb2, in_=hpb2_d[:, :])

            # persistent state
            relu_u = singles.tile([HID + 2, BL], dt.bfloat16)
            nc.sync.dma_start(out=relu_u[HID : HID + 2, :], in_=ones2_d[:, :])
            traj = singles.tile([2, SEQ * BL], dt.float32)
            lp0b = singles.tile([2, BL], dt.bfloat16)
            nc.sync.dma_start(out=lp0b, in_=lp0b_d[:, :])
            lp0f = singles.tile([2, BL], dt.float32)
            nc.sync.dma_start(out=lp0f, in_=lp0f_d[:, :])
            hb, cst = [], []
            for j in range(2):
                t_b = singles.tile([128, 512], dt.bfloat16, tag=f"hb{j}", name=f"hb{j}")
                nc.sync.dma_start(out=t_b, in_=h0b_d[j, :, :])
                hb.append(t_b)
                t_c = singles.tile([128, 512], dt.float32, tag=f"c{j}", name=f"c{j}")
                nc.sync.dma_start(out=t_c, in_=c0_d[j, :, :])
                cst.append(t_c)

            lp_bf = lp0b
            for t in range(SEQ):
                lp_f = lp0f[:, :] if t == 0 else traj[:2, ts(t - 1, BL)]

                # SE level 1: u = relu(A1.T @ lp + c1_se) -> bf16 rows 0..15
                u_ps = pss.tile([HID, BL], dt.float32, tag="u", name="u_ps")
                nc.tensor.matmul(u_ps, a1, lp_bf, start=True, stop=True)
                nc.scalar.activation(relu_u[0:HID, :], u_ps, ACTF.Relu, bias=c1se)

                # gates: 8 psum pairs, each [128, 512] = two m-tiles
                gate_sb = []
                for p in range(8):
                    ps = psg.tile([128, 2 * BL], dt.float32, tag="gp", name=f"gp{t}_{p}")
                    for half in range(2):
                        m = 2 * p + half
                        o_ap = ps[:, ts(half, BL)]
                        for kk in range(4):
                            rhs = hb[kk // 2][:, ts(kk % 2, BL)]
                            nc.tensor.matmul(
                                o_ap, whh[kk][:, ts(m, 128)], rhs,
                                start=(kk == 0), stop=False,
                            )
                        nc.tensor.matmul(
                            o_ap, w2t[:, ts(m, 128)], relu_u,
                            start=False, stop=True,
                        )
                    func = ACTF.Tanh if p in (4, 5) else ACTF.Sigmoid
                    gs = gpool.tile(
                        [128, 2 * BL], dt.float32, tag=f"gate{p % 2}", name=f"gate{t}_{p}"
                    )
                    nc.scalar.activation(gs, ps, func)
                    gate_sb.append(gs)

                # LSTM cell update (feature-half j)
                for j in range(2):
                    i_t, f_t, g_t, o_t = (
                        gate_sb[0 + j], gate_sb[2 + j], gate_sb[4 + j], gate_sb[6 + j],
                    )
                    t_ig = gpool.tile([128, 2 * BL], dt.float32, tag="tig", name=f"tig{t}_{j}")
                    t_fc = gpool.tile([128, 2 * BL], dt.float32, tag="tfc", name=f"tfc{t}_{j}")
                    nc.vector.tensor_mul(t_ig, i_t, g_t)
                    nc.vector.tensor_mul(t_fc, f_t, cst[j])
                    nc.vector.tensor_add(cst[j], t_fc, t_ig)
                    t_tc = gpool.tile([128, 2 * BL], dt.float32, tag="ttc", name=f"ttc{t}_{j}")
                    nc.scalar.activation(t_tc, cst[j], ACTF.Tanh)
                    nc.vector.tensor_mul(hb[j], o_t, t_tc)

                # HP head: v = relu(B1.T @ h + c1_hp); p = hp_w2.T @ v + lp
                v_ps = pss.tile([HID, BL], dt.float32, tag="v", name="v_ps")
                for kk in range(4):
                    rhs = hb[kk // 2][:, ts(kk % 2, BL)]
                    nc.tensor.matmul(v_ps, b1[kk], rhs, start=(kk == 0), stop=(kk == 3))
                r_hp = gpool.tile([HID, BL], dt.bfloat16, tag="rhp", name=f"rhp{t}")
                nc.scalar.activation(r_hp, v_ps, ACTF.Relu, bias=c1hp)
                p_ps = pss.tile([2, BL], dt.float32, tag="p", name="p_ps")
                nc.tensor.matmul(p_ps, hpw2, r_hp, start=True, stop=True)
                # lp carry add in fp32 on DVE, then sigmoid (f32 out + bf16 out)
                s_t = gpool.tile([2, BL], dt.float32, tag="st", name=f"st{t}")
                nc.vector.tensor_add(s_t, p_ps, lp_f)
                nc.scalar.activation(traj[:2, ts(t, BL)], s_t, ACTF.Sigmoid, bias=hpb2)
                lp_bf = gpool.tile([2, BL], dt.bfloat16, tag="lpb", name=f"lpb{t}")
                nc.scalar.activation(lp_bf, s_t, ACTF.Sigmoid, bias=hpb2)

            nc.sync.dma_start(
                out=traj_d[:, :, :].rearrange("p t b -> p (t b)"), in_=traj[:2, :]
            )
    patched = _fix_multiwait(nc.to_json_bytes())
    nc.to_json_bytes = lambda: patched
    return nc


def _pack_half(x_t):
    # [512, BL] feature-major -> [2, 128, 2*BL]: tile j holds feature-tiles
    # 2j (cols 0:BL) and 2j+1 (cols BL:2BL)
    xr = x_t.reshape(4, 128, BL)
    return np.stack(
        [np.concatenate([xr[2 * j], xr[2 * j + 1]], axis=1) for j in range(2)]
    )


def _host_prep(inputs):
    f = lambda k: np.asarray(inputs[k], dtype=np.float64)
    se_w1, se_b1 = f("se_w1"), f("se_b1")
    se_g, se_bt, se_m, se_v = f("se_g"), f("se_bt"), f("se_m"), f("se_v")
    se_w2, se_b2 = f("se_w2"), f("se_b2")
    w_ih, w_hh, b_ih, b_hh = f("w_ih"), f("w_hh"), f("b_ih"), f("b_hh")
    hp_w1, hp_b1 = f("hp_w1"), f("hp_b1")
    hp_g, hp_bt, hp_m, hp_v = f("hp_g"), f("hp_bt"), f("hp_m"), f("hp_v")
    hp_w2, hp_b2 = f("hp_w2"), f("hp_b2")

    s_se = se_g / np.sqrt(se_v + BN_EPS)
    a1 = (se_w1 * s_se[None, :]).astype(F32)
    c1_se = ((se_b1 - se_m) * s_se + se_bt).astype(F32)
    s_hp = hp_g / np.sqrt(hp_v + BN_EPS)
    b1 = (hp_w1 * s_hp[None, :]).astype(F32)
    c1_hp = ((hp_b1 - hp_m) * s_hp + hp_bt).astype(F32)

    w2t = (se_w2 @ w_ih.T).astype(F32)  # [16, 2048]
    b_eff = (b_ih + b_hh + w_ih @ se_b2).astype(F32)  # [2048]
    b_hi = b_eff.astype(BF16).astype(F32)
    b_lo = (b_eff - b_hi).astype(F32)
    w2t_ext = np.concatenate(
        [w2t, b_hi[None, :], b_lo[None, :]], axis=0
    ).astype(BF16)  # [18, 2048]

    rep = {
        "whhT": np.ascontiguousarray(w_hh.T.astype(F32).astype(BF16)).reshape(
            4, 128, NG
        ),
        "w2t": np.ascontiguousarray(w2t_ext),
        "b1": np.ascontiguousarray(b1.astype(BF16)).reshape(4, 128, HID),
        "a1": np.ascontiguousarray(a1.astype(BF16)),
        "hpw2": np.ascontiguousarray(hp_w2.astype(F32).astype(BF16)),
        "c1se": c1_se.reshape(HID, 1),
        "c1hp": c1_hp.reshape(HID, 1),
        "hpb2": hp_b2.astype(F32).reshape(2, 1),
        "ones2": np.ones((2, BL), dtype=BF16),
    }

    last_pos = np.asarray(inputs["last_pos"], dtype=F32)
    h0 = np.asarray(inputs["hh"], dtype=F32)[0]
    c0 = np.asarray(inputs["ch"], dtype=F32)[0]
    in_maps = []
    for c in range(NCORES):
        rows = slice(c * BL, (c + 1) * BL)
        h0t = np.ascontiguousarray(h0[rows].T)  # [512, BL]
        c0t = np.ascontiguousarray(c0[rows].T)
        m = dict(rep)
        lp0t = np.ascontiguousarray(last_pos[rows].T)  # [2, BL]
        m["lp0f"] = lp0t
        m["lp0b"] = lp0t.astype(BF16)
        m["h0b"] = _pack_half(h0t).astype(BF16)
        m["c0"] = _pack_half(c0t)
        in_maps.append(m)
    return in_maps


def _get_runner():
    """Build (once) a persistent jitted SPMD runner over 8 cores."""
    if "runner" in _CACHE:
        return _CACHE["runner"]

    import jax
    from jax.sharding import Mesh, PartitionSpec, NamedSharding
    from jax.experimental.shard_map import shard_map
    from concourse import bass2jax, mybir as _mb

    nc = _build_nc()
    bass2jax.install_neuronx_cc_hook()

    partition_name = nc.partition_id_tensor.name if nc.partition_id_tensor else None
    in_names, out_names, out_avals, zero_shapes = [], [], [], []
    for alloc in nc.m.functions[0].allocations:
        if not isinstance(alloc, _mb.MemoryLocationSet):
            continue
        name = alloc.memorylocations[0].name
        if alloc.kind == "ExternalInput":
            if name != partition_name:
                in_names.append(name)
        elif alloc.kind == "ExternalOutput":
            out_names.append(name)
            shape = tuple(alloc.tensor_shape)
            dtype = _mb.dt.np(alloc.dtype)
            out_avals.append(jax.core.ShapedArray(shape, dtype))
            zero_shapes.append((shape, dtype))
    n_params = len(in_names)
    all_names = in_names + out_names
    if partition_name is not None:
        all_names = all_names + [partition_name]
    donate = tuple(range(n_params, n_params + len(out_names)))

    def _body(*args):
        operands = list(args)
        if partition_name is not None:
            operands.append(bass2jax.partition_id_tensor())
        outs = bass2jax._bass_exec_p.bind(
            *operands,
            out_avals=tuple(out_avals),
            in_names=tuple(all_names),
            out_names=tuple(out_names),
            lowering_input_output_aliases=(),
            sim_require_finite=True,
            sim_require_nnan=True,
            nc=nc,
        )
        return tuple(outs)

    devices = jax.devices()[:NCORES]
    mesh = Mesh(np.asarray(devices), ("core",))
    spec = PartitionSpec("core")
    sharded = jax.jit(
        shard_map(
            _body,
            mesh=mesh,
            in_specs=(spec,) * (n_params + len(out_names)),
            out_specs=(spec,) * len(out_names),
            check_rep=False,
        ),
        donate_argnums=donate,
        keep_unused=True,
    )
    sharding = NamedSharding(mesh, spec)

    def stage(in_maps):
        """device_put concatenated inputs once; reusable across exec() calls."""
        concat = [
            np.concatenate([np.asarray(m[name]) for m in in_maps], axis=0)
            for name in in_names
        ]
        return [jax.device_put(a, sharding) for a in concat]

    def exec_(staged):
        zeros = [
            jax.device_put(np.zeros((NCORES * s[0], *s[1:]), d), sharding)
            for s, d in zero_shapes
        ]
        outs = sharded(*staged, *zeros)
        outs = [np.asarray(o) for o in outs]
        return {
            name: outs[i].reshape(NCORES, *out_avals[i].shape)
            for i, name in enumerate(out_names)
        }

    _CACHE["runner"] = (stage, exec_)
    return _CACHE["runner"]


def kernel(**inputs) -> np.ndarray:
    stage, exec_ = _get_runner()
    staged = stage(_host_prep(inputs))
    per_core = exec_(staged)["traj"]  # [8, 2, 32, BL]
    out = per_core.transpose(2, 0, 3, 1).reshape(SEQ, B, 2)
    return np.ascontiguousarray(out.astype(np.float32))


# revision 19
# speedup vs baseline: 123.9229x; 123.9229x over previous
"""Trainium2 Bass kernel for nn_Decoder (LSTM decoder with SE/HP MLP heads).

Strategy: pure data parallelism over batch (2048 -> 8 cores x 256).
Feature-major on-chip layout ([feature, batch]); weights stationary, batch
on the matmul moving dim. The SE MLP's output projection is folded into the
gate weights on the host (w2t = se_w2 @ w_ih.T), shrinking the x-part
contraction from K=512 to K=16, and all biases are folded into two extra
bf16 contraction rows (hi/lo split) so the gate bias is free.

Per step (32 sequential steps):
  u = relu(A1.T @ lp + c1_se)                      [16, 256]   (f32r matmul)
  gates = w_hh.T-tiles @ h_bf + W2T_ext @ u_ext    [2048, 256] (bf16, PSUM fp32)
  i,f,o = sigmoid(gates), g = tanh(gates)          (ACT, straight from PSUM)
  c = f*c + i*g; h = o*tanh(c)                     (DVE fp32, h also cast bf16)
  v = relu(B1.T @ h + c1_hp); p = hp_w2.T @ v + lp (f32r)
  lp = sigmoid(p + hp_b2)  -> traj[t]
"""

import json

import numpy as np
import ml_dtypes
from contextlib import ExitStack

import concourse.bass as bass
import concourse.mybir as mybir
import concourse.tile as tile
from concourse.bass import ts


def _fix_multiwait(bir_bytes: bytes) -> bytes:
    """Hoist excess sync waits onto injected EventSemaphore carriers
    (HW cap: 2 waits on EventSemaphore, 1 elsewhere; the Tile end-of-kernel
    drain can exceed this and the compiler rejects it)."""
    bir = json.loads(bir_bytes)
    for fn in bir.get("functions", []):
        for blk in fn.get("blocks", []):
            insts = blk.get("instructions")
            if not insts:
                continue
            out = []
            for inst in insts:
                si = inst.get("sync_info")
                waits = (si or {}).get("on_wait") or []
                cap = 2 if inst.get("opcode") == "EventSemaphore" else 1
                if len(waits) > cap:
                    excess, keep = waits[:-cap], waits[-cap:]
                    si["on_wait"] = keep
                    for i in range(0, len(excess), 2):
                        out.append({
                            "debug": inst.get("debug", 0),
                            "engine": inst["engine"],
                            "ins": [],
                            "name": f"{inst['name']}_xw{i}",
                            "opcode": "EventSemaphore",
                            "outs": [],
                            "sync_info": {"on_update": [], "on_wait": excess[i : i + 2]},
                        })
                out.append(inst)
            blk["instructions"] = out
    return json.dumps(bir).encode()

BF16 = ml_dtypes.bfloat16
F32 = np.float32

SEQ = 32
B = 2048
H = 512
E = 512
HID = 16
NCORES = 8
BL = B // NCORES  # 256 local batch
NG = 4 * H  # 2048 gate features
BN_EPS = 1e-5

_CACHE: dict = {}


def _build_nc(repeats: int = 1):
    nc = bass.Bass()
    dt = mybir.dt
    ACTF = mybir.ActivationFunctionType

    # --- DRAM tensors (per-core inputs; weights replicated across cores) ---
    whh_d = nc.dram_tensor("whhT", [4, 128, NG], dt.bfloat16, kind="ExternalInput")
    w2t_d = nc.dram_tensor("w2t", [HID + 2, NG], dt.bfloat16, kind="ExternalInput")
    b1_d = nc.dram_tensor("b1", [4, 128, HID], dt.bfloat16, kind="ExternalInput")
    a1_d = nc.dram_tensor("a1", [2, HID], dt.bfloat16, kind="ExternalInput")
    hpw2_d = nc.dram_tensor("hpw2", [HID, 2], dt.bfloat16, kind="ExternalInput")
    c1se_d = nc.dram_tensor("c1se", [HID, 1], dt.float32, kind="ExternalInput")
    c1hp_d = nc.dram_tensor("c1hp", [HID, 1], dt.float32, kind="ExternalInput")
    hpb2_d = nc.dram_tensor("hpb2", [2, 1], dt.float32, kind="ExternalInput")
    ones2_d = nc.dram_tensor("ones2", [2, BL], dt.bfloat16, kind="ExternalInput")
    lp0b_d = nc.dram_tensor("lp0b", [2, BL], dt.bfloat16, kind="ExternalInput")
    lp0f_d = nc.dram_tensor("lp0f", [2, BL], dt.float32, kind="ExternalInput")
    h0b_d = nc.dram_tensor("h0b", [2, 128, 512], dt.bfloat16, kind="ExternalInput")
    c0_d = nc.dram_tensor("c0", [2, 128, 512], dt.float32, kind="ExternalInput")
    traj_d = nc.dram_tensor("traj", [2, SEQ, BL], dt.float32, kind="ExternalOutput")

    with tile.TileContext(nc) as tc:
        with ExitStack() as ctx:
            singles = ctx.enter_context(tc.tile_pool(name="singles", bufs=1))
            gpool = ctx.enter_context(tc.tile_pool(name="gates", bufs=2))
            tpool = ctx.enter_context(tc.tile_pool(name="temps", bufs=3))
            psg = ctx.enter_context(tc.tile_pool(name="psg", bufs=7, space="PSUM"))
            pss = ctx.enter_context(tc.tile_pool(name="pss", bufs=1, space="PSUM"))

            # persistent weights
            whh = []
            for k in range(4):
                wt = singles.tile([128, NG], dt.bfloat16, tag=f"whh{k}", name=f"whh{k}")
                nc.sync.dma_start(out=wt, in_=whh_d[k, :, :])
                whh.append(wt)
            w2t = singles.tile([HID + 2, NG], dt.bfloat16)
            nc.sync.dma_start(out=w2t, in_=w2t_d[:, :])
            b1 = []
            for k in range(4):
                bt = singles.tile([128, HID], dt.bfloat16, tag=f"b1{k}", name=f"b1{k}")
                nc.sync.dma_start(out=bt, in_=b1_d[k, :, :])
                b1.append(bt)
            a1 = singles.tile([2, HID], dt.bfloat16)
            nc.sync.dma_start(out=a1, in_=a1_d[:, :])
            hpw2 = singles.tile([HID, 2], dt.bfloat16)
            nc.sync.dma_start(out=hpw2, in_=hpw2_d[:, :])
            c1se = singles.tile([HID, 1], dt.float32)
            nc.sync.dma_start(out=c1se, in_=c1se_d[:, :])
            c1hp = singles.tile([HID, 1], dt.float32)
            nc.sync.dma_start(out=c1hp, in_=c1hp_d[:, :])
            hpb2 = singles.tile([2, 1], dt.float32)
            nc.sync.dma_start(out=hpb2, in_=hpb2_d[:, :])

            # persistent state
            relu_u = singles.tile([HID + 2, BL], dt.bfloat16)
            nc.sync.dma_start(out=relu_u[HID : HID + 2, :], in_=ones2_d[:, :])
            traj = singles.tile([2, SEQ * BL], dt.float32)
            lp0b = singles.tile([2, BL], dt.bfloat16)
            nc.sync.dma_start(out=lp0b, in_=lp0b_d[:, :])
            lp0f = singles.tile([2, BL], dt.float32)
            nc.sync.dma_start(out=lp0f, in_=lp0f_d[:, :])
            hb, cst = [], []
            for j in range(2):
                t_b = singles.tile([128, 512], dt.bfloat16, tag=f"hb{j}", name=f"hb{j}")
                nc.sync.dma_start(out=t_b, in_=h0b_d[j, :, :])
                hb.append(t_b)
                t_c = singles.tile([128, 512], dt.float32, tag=f"c{j}", name=f"c{j}")
                nc.sync.dma_start(out=t_c, in_=c0_d[j, :, :])
                cst.append(t_c)

            def body():
                lp_bf = lp0b
                for t in range(SEQ):
                    step(t, lp_bf)
                    lp_bf = step.last_lp_bf
                if repeats > 1:
                    # restore loop-carried lp state for the next timing rep
                    nc.vector.tensor_copy(lp0f, traj[:2, ts(SEQ - 1, BL)])
                    nc.vector.tensor_copy(lp0b, traj[:2, ts(SEQ - 1, BL)])

            def step(t, lp_bf):
                lp_f = lp0f[:, :] if t == 0 else traj[:2, ts(t - 1, BL)]

                # SE level 1: u = relu(A1.T @ lp + c1_se) -> bf16 rows 0..15
                u_ps = pss.tile([HID, BL], dt.float32, tag="u", name="u_ps")
                nc.tensor.matmul(u_ps, a1, lp_bf, start=True, stop=True)
                nc.scalar.activation(relu_u[0:HID, :], u_ps, ACTF.Relu, bias=c1se)

                # gates: 8 psum pairs, each [128, 512] = two m-tiles
                gate_sb = []
                for p in range(8):
                    ps = psg.tile([128, 2 * BL], dt.float32, tag="gp", name=f"gp{t}_{p}")
                    for half in range(2):
                        m = 2 * p + half
                        o_ap = ps[:, ts(half, BL)]
                        for kk in range(4):
                            rhs = hb[kk // 2][:, ts(kk % 2, BL)]
                            nc.tensor.matmul(
                                o_ap, whh[kk][:, ts(m, 128)], rhs,
                                start=(kk == 0), stop=False,
                            )
                        nc.tensor.matmul(
                            o_ap, w2t[:, ts(m, 128)], relu_u,
                            start=False, stop=True,
                        )
                    func = ACTF.Tanh if p in (4, 5) else ACTF.Sigmoid
                    gs = gpool.tile(
                        [128, 2 * BL], dt.float32, tag=f"gate{p % 2}", name=f"gate{t}_{p}"
                    )
                    nc.scalar.activation(gs, ps, func)
                    gate_sb.append(gs)

                # LSTM cell update (feature-half j)
                for j in range(2):
                    i_t, f_t, g_t, o_t = (
                        gate_sb[0 + j], gate_sb[2 + j], gate_sb[4 + j], gate_sb[6 + j],
                    )
                    t_ig = gpool.tile([128, 2 * BL], dt.float32, tag="tig", name=f"tig{t}_{j}")
                    t_fc = gpool.tile([128, 2 * BL], dt.float32, tag="tfc", name=f"tfc{t}_{j}")
                    nc.vector.tensor_mul(t_ig, i_t, g_t)
                    nc.vector.tensor_mul(t_fc, f_t, cst[j])
                    nc.vector.tensor_add(cst[j], t_fc, t_ig)
                    t_tc = gpool.tile([128, 2 * BL], dt.float32, tag="ttc", name=f"ttc{t}_{j}")
                    nc.scalar.activation(t_tc, cst[j], ACTF.Tanh)
                    nc.vector.tensor_mul(hb[j], o_t, t_tc)

                # HP head: v = relu(B1.T @ h + c1_hp); p = hp_w2.T @ v + lp
                v_ps = pss.tile([HID, BL], dt.float32, tag="v", name="v_ps")
                for kk in range(4):
                    rhs = hb[kk // 2][:, ts(kk % 2, BL)]
                    nc.tensor.matmul(v_ps, b1[kk], rhs, start=(kk == 0), stop=(kk == 3))
                r_hp = gpool.tile([HID, BL], dt.bfloat16, tag="rhp", name=f"rhp{t}")
                nc.scalar.activation(r_hp, v_ps, ACTF.Relu, bias=c1hp)
                p_ps = pss.tile([2, BL], dt.float32, tag="p", name="p_ps")
                nc.tensor.matmul(p_ps, hpw2, r_hp, start=True, stop=True)
                # lp carry add in fp32 on DVE, then sigmoid (f32 out + bf16 out)
                s_t = gpool.tile([2, BL], dt.float32, tag="st", name=f"st{t}")
                nc.vector.tensor_add(s_t, p_ps, lp_f)
                nc.scalar.activation(traj[:2, ts(t, BL)], s_t, ACTF.Sigmoid, bias=hpb2)
                lp_bf = gpool.tile([2, BL], dt.bfloat16, tag="lpb", name=f"lpb{t}")
                nc.scalar.activation(lp_bf, s_t, ACTF.Sigmoid, bias=hpb2)
                step.last_lp_bf = lp_bf

            if repeats == 1:
                body()
            else:
                with tc.For_i(0, repeats, 1):
                    body()

            nc.sync.dma_start(
                out=traj_d[:, :, :].rearrange("p t b -> p (t b)"), in_=traj[:2, :]
            )
    patched = _fix_multiwait(nc.to_json_bytes())
    nc.to_json_bytes = lambda: patched
    return nc


def _pack_half(x_t):
    # [512, BL] feature-major -> [2, 128, 2*BL]: tile j holds feature-tiles
    # 2j (cols 0:BL) and 2j+1 (cols BL:2BL)
    xr = x_t.reshape(4, 128, BL)
    return np.stack(
        [np.concatenate([xr[2 * j], xr[2 * j + 1]], axis=1) for j in range(2)]
    )


def _host_prep(inputs):
    f = lambda k: np.asarray(inputs[k], dtype=np.float64)
    se_w1, se_b1 = f("se_w1"), f("se_b1")
    se_g, se_bt, se_m, se_v = f("se_g"), f("se_bt"), f("se_m"), f("se_v")
    se_w2, se_b2 = f("se_w2"), f("se_b2")
    w_ih, w_hh, b_ih, b_hh = f("w_ih"), f("w_hh"), f("b_ih"), f("b_hh")
    hp_w1, hp_b1 = f("hp_w1"), f("hp_b1")
    hp_g, hp_bt, hp_m, hp_v = f("hp_g"), f("hp_bt"), f("hp_m"), f("hp_v")
    hp_w2, hp_b2 = f("hp_w2"), f("hp_b2")

    s_se = se_g / np.sqrt(se_v + BN_EPS)
    a1 = (se_w1 * s_se[None, :]).astype(F32)
    c1_se = ((se_b1 - se_m) * s_se + se_bt).astype(F32)
    s_hp = hp_g / np.sqrt(hp_v + BN_EPS)
    b1 = (hp_w1 * s_hp[None, :]).astype(F32)
    c1_hp = ((hp_b1 - hp_m) * s_hp + hp_bt).astype(F32)

    w2t = (se_w2 @ w_ih.T).astype(F32)  # [16, 2048]
    b_eff = (b_ih + b_hh + w_ih @ se_b2).astype(F32)  # [2048]
    b_hi = b_eff.astype(BF16).astype(F32)
    b_lo = (b_eff - b_hi).astype(F32)
    w2t_ext = np.concatenate(
        [w2t, b_hi[None, :], b_lo[None, :]], axis=0
    ).astype(BF16)  # [18, 2048]

    rep = {
        "whhT": np.ascontiguousarray(w_hh.T.astype(F32).astype(BF16)).reshape(
            4, 128, NG
        ),
        "w2t": np.ascontiguousarray(w2t_ext),
        "b1": np.ascontiguousarray(b1.astype(BF16)).reshape(4, 128, HID),
        "a1": np.ascontiguousarray(a1.astype(BF16)),
        "hpw2": np.ascontiguousarray(hp_w2.astype(F32).astype(BF16)),
        "c1se": c1_se.reshape(HID, 1),
        "c1hp": c1_hp.reshape(HID, 1),
        "hpb2": hp_b2.astype(F32).reshape(2, 1),
        "ones2": np.ones((2, BL), dtype=BF16),
    }

    last_pos = np.asarray(inputs["last_pos"], dtype=F32)
    h0 = np.asarray(inputs["hh"], dtype=F32)[0]
    c0 = np.asarray(inputs["ch"], dtype=F32)[0]
    in_maps = []
    for c in range(NCORES):
        rows = slice(c * BL, (c + 1) * BL)
        h0t = np.ascontiguousarray(h0[rows].T)  # [512, BL]
        c0t = np.ascontiguousarray(c0[rows].T)
        m = dict(rep)
        lp0t = np.ascontiguousarray(last_pos[rows].T)  # [2, BL]
        m["lp0f"] = lp0t
        m["lp0b"] = lp0t.astype(BF16)
        m["h0b"] = _pack_half(h0t).astype(BF16)
        m["c0"] = _pack_half(c0t)
        in_maps.append(m)
    return in_maps


def _get_runner(repeats: int = 1):
    """Build (once) a persistent jitted SPMD runner over 8 cores."""
    key = ("runner", repeats)
    if key in _CACHE:
        return _CACHE[key]

    import jax
    from jax.sharding import Mesh, PartitionSpec, NamedSharding
    from jax.experimental.shard_map import shard_map
    from concourse import bass2jax, mybir as _mb

    nc = _build_nc(repeats)
    bass2jax.install_neuronx_cc_hook()

    partition_name = nc.partition_id_tensor.name if nc.partition_id_tensor else None
    in_names, out_names, out_avals, zero_shapes = [], [], [], []
    for alloc in nc.m.functions[0].allocations:
        if not isinstance(alloc, _mb.MemoryLocationSet):
            continue
        name = alloc.memorylocations[0].name
        if alloc.kind == "ExternalInput":
            if name != partition_name:
                in_names.append(name)
        elif alloc.kind == "ExternalOutput":
            out_names.append(name)
            shape = tuple(alloc.tensor_shape)
            dtype = _mb.dt.np(alloc.dtype)
            out_avals.append(jax.core.ShapedArray(shape, dtype))
            zero_shapes.append((shape, dtype))
    n_params = len(in_names)
    all_names = in_names + out_names
    if partition_name is not None:
        all_names = all_names + [partition_name]
    donate = tuple(range(n_params, n_params + len(out_names)))

    def _body(*args):
        operands = list(args)
        if partition_name is not None:
            operands.append(bass2jax.partition_id_tensor())
        outs = bass2jax._bass_exec_p.bind(
            *operands,
            out_avals=tuple(out_avals),
            in_names=tuple(all_names),
            out_names=tuple(out_names),
            lowering_input_output_aliases=(),
            sim_require_finite=True,
            sim_require_nnan=True,
            nc=nc,
        )
        return tuple(outs)

    devices = jax.devices()[:NCORES]
    mesh = Mesh(np.asarray(devices), ("core",))
    spec = PartitionSpec("core")
    sharded = jax.jit(
        shard_map(
            _body,
            mesh=mesh,
            in_specs=(spec,) * (n_params + len(out_names)),
            out_specs=(spec,) * len(out_names),
            check_rep=False,
        ),
        donate_argnums=donate,
        keep_unused=True,
    )
    sharding = NamedSharding(mesh, spec)

    def stage(in_maps):
        """device_put concatenated inputs once; reusable across exec() calls."""
        concat = [
            np.concatenate([np.asarray(m[name]) for m in in_maps], axis=0)
            for name in in_names
        ]
        return [jax.device_put(a, sharding) for a in concat]

    def exec_(staged):
        zeros = [
            jax.device_put(np.zeros((NCORES * s[0], *s[1:]), d), sharding)
            for s, d in zero_shapes
        ]
        outs = sharded(*staged, *zeros)
        outs = [np.asarray(o) for o in outs]
        return {
            name: outs[i].reshape(NCORES, *out_avals[i].shape)
            for i, name in enumerate(out_names)
        }

    _CACHE[key] = (stage, exec_)
    return _CACHE[key]


def kernel(**inputs) -> np.ndarray:
    stage, exec_ = _get_runner()
    staged = stage(_host_prep(inputs))
    per_core = exec_(staged)["traj"]  # [8, 2, 32, BL]
    out = per_core.transpose(2, 0, 3, 1).reshape(SEQ, B, 2)
    return np.ascontiguousarray(out.astype(np.float32))


# revision 25
# speedup vs baseline: 189.1819x; 1.5266x over previous
"""Trainium2 Bass kernel for nn_Decoder (LSTM decoder with SE/HP MLP heads).

Strategy: pure data parallelism over batch (2048 -> 8 cores x 256).
Feature-major on-chip layout ([feature, batch]); weights stationary, batch
on the matmul moving dim. The SE MLP's output projection is folded into the
gate weights on the host (w2t = se_w2 @ w_ih.T), shrinking the x-part
contraction from K=512 to K=16, and all biases are folded into two extra
bf16 contraction rows (hi/lo split) so the gate bias is free.

Per step (32 sequential steps):
  u = relu(A1.T @ lp + c1_se)                      [16, 256]   (f32r matmul)
  gates = w_hh.T-tiles @ h_bf + W2T_ext @ u_ext    [2048, 256] (bf16, PSUM fp32)
  i,f,o = sigmoid(gates), g = tanh(gates)          (ACT, straight from PSUM)
  c = f*c + i*g; h = o*tanh(c)                     (DVE fp32, h also cast bf16)
  v = relu(B1.T @ h + c1_hp); p = hp_w2.T @ v + lp (f32r)
  lp = sigmoid(p + hp_b2)  -> traj[t]
"""

import json

import numpy as np
import ml_dtypes
from contextlib import ExitStack

import concourse.bass as bass
import concourse.mybir as mybir
import concourse.tile as tile
from concourse.bass import ts


def _fix_multiwait(bir_bytes: bytes) -> bytes:
    """Hoist excess sync waits onto injected EventSemaphore carriers
    (HW cap: 2 waits on EventSemaphore, 1 elsewhere; the Tile end-of-kernel
    drain can exceed this and the compiler rejects it)."""
    bir = json.loads(bir_bytes)
    for fn in bir.get("functions", []):
        for blk in fn.get("blocks", []):
            insts = blk.get("instructions")
            if not insts:
                continue
            out = []
            for inst in insts:
                si = inst.get("sync_info")
                waits = (si or {}).get("on_wait") or []
                cap = 2 if inst.get("opcode") == "EventSemaphore" else 1
                if len(waits) > cap:
                    excess, keep = waits[:-cap], waits[-cap:]
                    si["on_wait"] = keep
                    for i in range(0, len(excess), 2):
                        out.append({
                            "debug": inst.get("debug", 0),
                            "engine": inst["engine"],
                            "ins": [],
                            "name": f"{inst['name']}_xw{i}",
                            "opcode": "EventSemaphore",
                            "outs": [],
                            "sync_info": {"on_update": [], "on_wait": excess[i : i + 2]},
                        })
                out.append(inst)
            blk["instructions"] = out
    return json.dumps(bir).encode()

BF16 = ml_dtypes.bfloat16
F32 = np.float32

SEQ = 32
B = 2048
H = 512
E = 512
HID = 16
NCORES = 8
BL = B // NCORES  # 256 local batch
NG = 4 * H  # 2048 gate features
BN_EPS = 1e-5

_CACHE: dict = {}


def _build_nc(repeats: int = 1):
    nc = bass.Bass()
    dt = mybir.dt
    ACTF = mybir.ActivationFunctionType

    # --- DRAM tensors (per-core inputs; weights replicated across cores) ---
    whh_d = nc.dram_tensor("whhT", [4, 128, NG], dt.bfloat16, kind="ExternalInput")
    w2t_d = nc.dram_tensor("w2t", [HID + 2, NG], dt.bfloat16, kind="ExternalInput")
    b1_d = nc.dram_tensor("b1", [4, 128, HID], dt.bfloat16, kind="ExternalInput")
    a1_d = nc.dram_tensor("a1", [2, HID], dt.bfloat16, kind="ExternalInput")
    hpw2_d = nc.dram_tensor("hpw2", [HID, 2], dt.bfloat16, kind="ExternalInput")
    c1se_d = nc.dram_tensor("c1se", [HID, 1], dt.float32, kind="ExternalInput")
    c1hp_d = nc.dram_tensor("c1hp", [HID, 1], dt.float32, kind="ExternalInput")
    hpb2_d = nc.dram_tensor("hpb2", [2, 1], dt.float32, kind="ExternalInput")
    ones2_d = nc.dram_tensor("ones2", [2, BL], dt.bfloat16, kind="ExternalInput")
    lp0b_d = nc.dram_tensor("lp0b", [2, BL], dt.bfloat16, kind="ExternalInput")
    lp0f_d = nc.dram_tensor("lp0f", [2, BL], dt.float32, kind="ExternalInput")
    h0b_d = nc.dram_tensor("h0b", [2, 128, 512], dt.bfloat16, kind="ExternalInput")
    c0_d = nc.dram_tensor("c0", [2, 128, 512], dt.float32, kind="ExternalInput")
    traj_d = nc.dram_tensor("traj", [2, SEQ, BL], dt.float32, kind="ExternalOutput")

    with tile.TileContext(nc) as tc:
        with ExitStack() as ctx:
            singles = ctx.enter_context(tc.tile_pool(name="singles", bufs=1))
            gpool = ctx.enter_context(tc.tile_pool(name="gates", bufs=2))
            tpool = ctx.enter_context(tc.tile_pool(name="temps", bufs=3))
            psg = ctx.enter_context(tc.tile_pool(name="psg", bufs=7, space="PSUM"))
            pss = ctx.enter_context(tc.tile_pool(name="pss", bufs=1, space="PSUM"))

            # persistent weights
            whh = []
            for k in range(4):
                wt = singles.tile([128, NG], dt.bfloat16, tag=f"whh{k}", name=f"whh{k}")
                nc.sync.dma_start(out=wt, in_=whh_d[k, :, :])
                whh.append(wt)
            w2t = singles.tile([HID + 2, NG], dt.bfloat16)
            nc.sync.dma_start(out=w2t, in_=w2t_d[:, :])
            b1 = []
            for k in range(4):
                bt = singles.tile([128, HID], dt.bfloat16, tag=f"b1{k}", name=f"b1{k}")
                nc.sync.dma_start(out=bt, in_=b1_d[k, :, :])
                b1.append(bt)
            a1 = singles.tile([2, HID], dt.bfloat16)
            nc.sync.dma_start(out=a1, in_=a1_d[:, :])
            hpw2 = singles.tile([HID, 2], dt.bfloat16)
            nc.sync.dma_start(out=hpw2, in_=hpw2_d[:, :])
            c1se = singles.tile([HID, 1], dt.float32)
            nc.sync.dma_start(out=c1se, in_=c1se_d[:, :])
            c1hp = singles.tile([HID, 1], dt.float32)
            nc.sync.dma_start(out=c1hp, in_=c1hp_d[:, :])
            hpb2 = singles.tile([2, 1], dt.float32)
            nc.sync.dma_start(out=hpb2, in_=hpb2_d[:, :])

            # persistent state
            relu_u = singles.tile([HID + 2, BL], dt.bfloat16)
            nc.sync.dma_start(out=relu_u[HID : HID + 2, :], in_=ones2_d[:, :])
            traj = singles.tile([2, SEQ * BL], dt.float32)
            lp0b = singles.tile([2, BL], dt.bfloat16)
            nc.sync.dma_start(out=lp0b, in_=lp0b_d[:, :])
            lp0f = singles.tile([2, BL], dt.float32)
            nc.sync.dma_start(out=lp0f, in_=lp0f_d[:, :])
            # h is ping-pong buffered: step t reads hb2[t%2], writes hb2[(t+1)%2]
            hb2, cst = {0: [], 1: []}, []
            for j in range(2):
                t_b = singles.tile([128, 512], dt.bfloat16, tag=f"hbA{j}", name=f"hbA{j}")
                nc.sync.dma_start(out=t_b, in_=h0b_d[j, :, :])
                hb2[0].append(t_b)
                t_b2 = singles.tile([128, 512], dt.bfloat16, tag=f"hbB{j}", name=f"hbB{j}")
                hb2[1].append(t_b2)
                t_c = singles.tile([128, 512], dt.float32, tag=f"c{j}", name=f"c{j}")
                nc.sync.dma_start(out=t_c, in_=c0_d[j, :, :])
                cst.append(t_c)

            pairs: dict = {}

            def hwave_part(t, p, kks):
                """Emit h-dependent matmuls (contraction tiles `kks`) for pair
                p's HALF 0 only — at most one open accumulation group per PSUM
                zero-region (a second start=True in the region while a group
                is pending corrupts the pending half's accumulation)."""
                ps = pairs.get((t, p))
                if ps is None:
                    ps = psg.tile([128, 2 * BL], dt.float32, tag="gp", name=f"gp{t}_{p}")
                    pairs[(t, p)] = ps
                cur = hb2[t % 2]
                m = 2 * p
                for kk in kks:
                    nc.tensor.matmul(
                        ps[:, ts(0, BL)],
                        whh[kk][:, ts(m, 128)],
                        cur[kk // 2][:, ts(kk % 2, BL)],
                        start=(kk == 0), stop=False,
                    )
                return ps

            def xclose(t, p):
                """Close half 0 with the SE x-part (K=18, bias folded in),
                run half 1 as a complete sequential group, then evacuate via
                fused sigmoid/tanh."""
                ps = pairs.pop((t, p))
                cur = hb2[t % 2]
                nc.tensor.matmul(
                    ps[:, ts(0, BL)], w2t[:, ts(2 * p, 128)], relu_u,
                    start=False, stop=True,
                )
                m = 2 * p + 1
                for kk in range(4):
                    nc.tensor.matmul(
                        ps[:, ts(1, BL)],
                        whh[kk][:, ts(m, 128)],
                        cur[kk // 2][:, ts(kk % 2, BL)],
                        start=(kk == 0), stop=False,
                    )
                nc.tensor.matmul(
                    ps[:, ts(1, BL)], w2t[:, ts(m, 128)], relu_u,
                    start=False, stop=True,
                )
                func = ACTF.Tanh if p in (4, 5) else ACTF.Sigmoid
                gs = gpool.tile(
                    [128, 2 * BL], dt.float32, tag=f"gate{p}", name=f"gate{t}_{p}"
                )
                nc.scalar.activation(gs, ps, func)
                return gs

            def elem(t, j, gs):
                """LSTM cell update for feature-half j; writes c in place and
                the NEXT ping-pong h buffer."""
                i_t, f_t, g_t, o_t = gs[0 + j], gs[2 + j], gs[4 + j], gs[6 + j]
                t_ig = tpool.tile([128, 2 * BL], dt.float32, tag="tig", name=f"tig{t}_{j}")
                t_fc = tpool.tile([128, 2 * BL], dt.float32, tag="tfc", name=f"tfc{t}_{j}")
                nc.vector.tensor_mul(t_ig, i_t, g_t)
                nc.vector.tensor_mul(t_fc, f_t, cst[j])
                nc.vector.tensor_add(cst[j], t_fc, t_ig)
                t_tc = tpool.tile([128, 2 * BL], dt.float32, tag="ttc", name=f"ttc{t}_{j}")
                nc.scalar.activation(t_tc, cst[j], ACTF.Tanh)
                nc.vector.tensor_mul(hb2[(t + 1) % 2][j], o_t, t_tc)

            def body():
                lp_bf = lp0b
                # step-0 prologue (normally done in the previous step's tail)
                hwave_part(0, 0, (0, 1, 2, 3))
                hwave_part(0, 2, (0, 1, 2, 3))
                u_ps = pss.tile([HID, BL], dt.float32, tag="small", name="u_0")
                nc.tensor.matmul(u_ps, a1, lp_bf, start=True, stop=True)
                hwave_part(0, 4, (0, 1, 2, 3))
                hwave_part(0, 6, (0, 1, 2, 3))
                hwave_part(0, 1, (0, 1, 2, 3))
                hwave_part(0, 3, (0, 1, 2, 3))
                nc.scalar.activation(relu_u[0:HID, :], u_ps, ACTF.Relu, bias=c1se)

                for t in range(SEQ):
                    lp_f = lp0f[:, :] if t == 0 else traj[:2, ts(t - 1, BL)]
                    nxt = t + 1 if t + 1 < SEQ else None
                    gs = {}
                    # j=0 pairs first so its elementwise chain overlaps j=1 MMs
                    for p in (0, 2, 4, 6):
                        gs[p] = xclose(t, p)
                    elem(t, 0, gs)
                    for p in (1, 3):
                        gs[p] = xclose(t, p)
                    for p in (5, 7):
                        hwave_part(t, p, (0, 1, 2, 3))
                        gs[p] = xclose(t, p)
                    elem(t, 1, gs)

                    # tail: HP head chain interleaved with next-step fill waves
                    nh = hb2[(t + 1) % 2]
                    v_ps = pss.tile([HID, BL], dt.float32, tag="small", name=f"v{t}")
                    for kk in (0, 1):
                        nc.tensor.matmul(v_ps, b1[kk], nh[0][:, ts(kk, BL)],
                                         start=(kk == 0), stop=False)
                    if nxt is not None:
                        for p in (0, 2, 4, 6):
                            hwave_part(nxt, p, (0, 1))
                    for kk in (2, 3):
                        nc.tensor.matmul(v_ps, b1[kk], nh[1][:, ts(kk - 2, BL)],
                                         start=False, stop=(kk == 3))
                    r_hp = tpool.tile([HID, BL], dt.bfloat16, tag="rhp", name=f"rhp{t}")
                    nc.scalar.activation(r_hp, v_ps, ACTF.Relu, bias=c1hp)
                    if nxt is not None:
                        hwave_part(nxt, 0, (2, 3))
                        hwave_part(nxt, 2, (2, 3))
                    p_ps = pss.tile([HID, BL], dt.float32, tag="small", name=f"p{t}")
                    nc.tensor.matmul(p_ps[:2, :], hpw2, r_hp, start=True, stop=True)
                    if nxt is not None:
                        hwave_part(nxt, 4, (2, 3))
                    # lp carry add in fp32 on DVE, then sigmoid (f32 + bf16 out)
                    s_t = tpool.tile([2, BL], dt.float32, tag="st", name=f"st{t}")
                    nc.vector.tensor_add(s_t, p_ps[:2, :], lp_f)
                    nc.scalar.activation(traj[:2, ts(t, BL)], s_t, ACTF.Sigmoid, bias=hpb2)
                    lp_bf = tpool.tile([2, BL], dt.bfloat16, tag="lpb", name=f"lpb{t}")
                    nc.scalar.activation(lp_bf, s_t, ACTF.Sigmoid, bias=hpb2)
                    if nxt is not None:
                        u_ps = pss.tile([HID, BL], dt.float32, tag="small", name=f"u{nxt}")
                        nc.tensor.matmul(u_ps, a1, lp_bf, start=True, stop=True)
                        hwave_part(nxt, 6, (2, 3))
                        hwave_part(nxt, 1, (0, 1, 2, 3))
                        hwave_part(nxt, 3, (0, 1, 2, 3))
                        nc.scalar.activation(relu_u[0:HID, :], u_ps, ACTF.Relu, bias=c1se)

                if repeats > 1:
                    # restore loop-carried lp state for the next timing rep
                    nc.vector.tensor_copy(lp0f, traj[:2, ts(SEQ - 1, BL)])
                    nc.vector.tensor_copy(lp0b, traj[:2, ts(SEQ - 1, BL)])

            if repeats == 1:
                body()
            else:
                with tc.For_i(0, repeats, 1):
                    body()

            nc.sync.dma_start(
                out=traj_d[:, :, :].rearrange("p t b -> p (t b)"), in_=traj[:2, :]
            )
    patched = _fix_multiwait(nc.to_json_bytes())
    nc.to_json_bytes = lambda: patched
    return nc


def _pack_half(x_t):
    # [512, BL] feature-major -> [2, 128, 2*BL]: tile j holds feature-tiles
    # 2j (cols 0:BL) and 2j+1 (cols BL:2BL)
    xr = x_t.reshape(4, 128, BL)
    return np.stack(
        [np.concatenate([xr[2 * j], xr[2 * j + 1]], axis=1) for j in range(2)]
    )


def _host_prep(inputs):
    f = lambda k: np.asarray(inputs[k], dtype=np.float64)
    se_w1, se_b1 = f("se_w1"), f("se_b1")
    se_g, se_bt, se_m, se_v = f("se_g"), f("se_bt"), f("se_m"), f("se_v")
    se_w2, se_b2 = f("se_w2"), f("se_b2")
    w_ih, w_hh, b_ih, b_hh = f("w_ih"), f("w_hh"), f("b_ih"), f("b_hh")
    hp_w1, hp_b1 = f("hp_w1"), f("hp_b1")
    hp_g, hp_bt, hp_m, hp_v = f("hp_g"), f("hp_bt"), f("hp_m"), f("hp_v")
    hp_w2, hp_b2 = f("hp_w2"), f("hp_b2")

    s_se = se_g / np.sqrt(se_v + BN_EPS)
    a1 = (se_w1 * s_se[None, :]).astype(F32)
    c1_se = ((se_b1 - se_m) * s_se + se_bt).astype(F32)
    s_hp = hp_g / np.sqrt(hp_v + BN_EPS)
    b1 = (hp_w1 * s_hp[None, :]).astype(F32)
    c1_hp = ((hp_b1 - hp_m) * s_hp + hp_bt).astype(F32)

    w2t = (se_w2 @ w_ih.T).astype(F32)  # [16, 2048]
    b_eff = (b_ih + b_hh + w_ih @ se_b2).astype(F32)  # [2048]
    b_hi = b_eff.astype(BF16).astype(F32)
    b_lo = (b_eff - b_hi).astype(F32)
    w2t_ext = np.concatenate(
        [w2t, b_hi[None, :], b_lo[None, :]], axis=0
    ).astype(BF16)  # [18, 2048]

    rep = {
        "whhT": np.ascontiguousarray(w_hh.T.astype(F32).astype(BF16)).reshape(
            4, 128, NG
        ),
        "w2t": np.ascontiguousarray(w2t_ext),
        "b1": np.ascontiguousarray(b1.astype(BF16)).reshape(4, 128, HID),
        "a1": np.ascontiguousarray(a1.astype(BF16)),
        "hpw2": np.ascontiguousarray(hp_w2.astype(F32).astype(BF16)),
        "c1se": c1_se.reshape(HID, 1),
        "c1hp": c1_hp.reshape(HID, 1),
        "hpb2": hp_b2.astype(F32).reshape(2, 1),
        "ones2": np.ones((2, BL), dtype=BF16),
    }

    last_pos = np.asarray(inputs["last_pos"], dtype=F32)
    h0 = np.asarray(inputs["hh"], dtype=F32)[0]
    c0 = np.asarray(inputs["ch"], dtype=F32)[0]
    in_maps = []
    for c in range(NCORES):
        rows = slice(c * BL, (c + 1) * BL)
        h0t = np.ascontiguousarray(h0[rows].T)  # [512, BL]
        c0t = np.ascontiguousarray(c0[rows].T)
        m = dict(rep)
        lp0t = np.ascontiguousarray(last_pos[rows].T)  # [2, BL]
        m["lp0f"] = lp0t
        m["lp0b"] = lp0t.astype(BF16)
        m["h0b"] = _pack_half(h0t).astype(BF16)
        m["c0"] = _pack_half(c0t)
        in_maps.append(m)
    return in_maps


def _get_runner(repeats: int = 1):
    """Build (once) a persistent jitted SPMD runner over 8 cores."""
    key = ("runner", repeats)
    if key in _CACHE:
        return _CACHE[key]

    import jax
    from jax.sharding import Mesh, PartitionSpec, NamedSharding
    from jax.experimental.shard_map import shard_map
    from concourse import bass2jax, mybir as _mb

    nc = _build_nc(repeats)
    bass2jax.install_neuronx_cc_hook()

    partition_name = nc.partition_id_tensor.name if nc.partition_id_tensor else None
    in_names, out_names, out_avals, zero_shapes = [], [], [], []
    for alloc in nc.m.functions[0].allocations:
        if not isinstance(alloc, _mb.MemoryLocationSet):
            continue
        name = alloc.memorylocations[0].name
        if alloc.kind == "ExternalInput":
            if name != partition_name:
                in_names.append(name)
        elif alloc.kind == "ExternalOutput":
            out_names.append(name)
            shape = tuple(alloc.tensor_shape)
            dtype = _mb.dt.np(alloc.dtype)
            out_avals.append(jax.core.ShapedArray(shape, dtype))
            zero_shapes.append((shape, dtype))
    n_params = len(in_names)
    all_names = in_names + out_names
    if partition_name is not None:
        all_names = all_names + [partition_name]
    donate = tuple(range(n_params, n_params + len(out_names)))

    def _body(*args):
        operands = list(args)
        if partition_name is not None:
            operands.append(bass2jax.partition_id_tensor())
        outs = bass2jax._bass_exec_p.bind(
            *operands,
            out_avals=tuple(out_avals),
            in_names=tuple(all_names),
            out_names=tuple(out_names),
            lowering_input_output_aliases=(),
            sim_require_finite=True,
            sim_require_nnan=True,
            nc=nc,
        )
        return tuple(outs)

    devices = jax.devices()[:NCORES]
    mesh = Mesh(np.asarray(devices), ("core",))
    spec = PartitionSpec("core")
    sharded = jax.jit(
        shard_map(
            _body,
            mesh=mesh,
            in_specs=(spec,) * (n_params + len(out_names)),
            out_specs=(spec,) * len(out_names),
            check_rep=False,
        ),
        donate_argnums=donate,
        keep_unused=True,
    )
    sharding = NamedSharding(mesh, spec)

    def stage(in_maps):
        """device_put concatenated inputs once; reusable across exec() calls."""
        concat = [
            np.concatenate([np.asarray(m[name]) for m in in_maps], axis=0)
            for name in in_names
        ]
        return [jax.device_put(a, sharding) for a in concat]

    def exec_(staged):
        zeros = [
            jax.device_put(np.zeros((NCORES * s[0], *s[1:]), d), sharding)
            for s, d in zero_shapes
        ]
        outs = sharded(*staged, *zeros)
        outs = [np.asarray(o) for o in outs]
        return {
            name: outs[i].reshape(NCORES, *out_avals[i].shape)
            for i, name in enumerate(out_names)
        }

    _CACHE[key] = (stage, exec_)
    return _CACHE[key]


def kernel(**inputs) -> np.ndarray:
    stage, exec_ = _get_runner()
    staged = stage(_host_prep(inputs))
    per_core = exec_(staged)["traj"]  # [8, 2, 32, BL]
    out = per_core.transpose(2, 0, 3, 1).reshape(SEQ, B, 2)
    return np.ascontiguousarray(out.astype(np.float32))
